# revision 2
# baseline (speedup 1.0000x reference)
"""MoE (noisy top-2 routing, 8 experts) on 8 Trainium2 NeuronCores — v3.

Strategy (expert-parallel, per sharding hint):
  Phase 1 (device, 8-way data-parallel over tokens): gating network
      h = x@Wg+bg + noise * softplus(x@Wn+bn), top-2 via DVE max8/max_index.
      Route ships (ix1, ix2, v1, v2) per token; the 2-way softmax over the
      top-2 logits is applied on host during dispatch.
  Host: dispatch — gather each expert's tokens by top-2 expert id.
  Phase 2 (device, 8-way expert-parallel, bf16): per-expert FFN
      y = (relu(x@W1+b1)@W2 + b2) * gate  on that expert's tokens.
      Stage A streams W1 slabs and materializes hid for the full H;
      Stage B contracts the full H per output d-tile in PSUM.
  Host: combine — scatter-add per-expert outputs back to token order.

v3 changes: phase 1 rewritten (fewer/bigger DMAs, clean ACT queue with a
single combined Exp+Ln table load, direct [128,16] route output); phase 2
keeps the Scalar queue free of DMA triggers, orders input DMAs strictly by
consumption, and runs at cap=1064 (actual max expert load is 1061).
"""
import sys

sys.path.insert(0, "/opt/trn_rl_repo")
import ml_dtypes
import numpy as np
import concourse.bass as bass  # noqa: F401
from concourse import bacc
import concourse.mybir as mybir
import concourse.tile as tile
from concourse.bass_utils import run_bass_kernel_spmd
from concourse.masks import make_identity

N_CORES = 8
B, S, D, H, E = 2, 2048, 768, 3072, 8
T = B * S            # 4096 tokens
T1 = T // N_CORES    # 512 tokens per core in phase 1
KD = D // 128        # 6 contraction chunks over D
NT = T1 // 128       # 4 token tiles per core in phase 1
CAP = 1064           # per-expert token capacity (actual max load is 1061)
HSLAB = 768          # W1 h-slab per stage-A iteration
NSLAB = H // HSLAB   # 4
KH = HSLAB // 128    # 6 h-chunks per slab
ND = D // 128        # 6 output d-tiles
WGN = 64             # gating lhsT cols: gate rows 0..7, noise rows 32..39
NLE_SET = 6          # act_func_sets id of natural_log_exp_and_others

F32 = mybir.dt.float32
F32R = mybir.dt.float32r
BF16 = mybir.dt.bfloat16
U32 = mybir.dt.uint32
AF = mybir.ActivationFunctionType
BFNP = ml_dtypes.bfloat16

_cache = {}
last_perf = {}


def _chunks_for(cap):
    """Split cap into matmul moving-dim chunks: each ≤512 (PSUM bank),
    multiples of 8, as equal as possible (all ≥236 keeps LDW hidden)."""
    n = -(-cap // 512)
    base = cap // n // 8 * 8
    sizes = [base] * n
    rem = cap - base * n
    i = 0
    while rem > 0:
        add = min(8, rem, 512 - sizes[i])
        sizes[i] += add
        rem -= add
        i = (i + 1) % n
    offs = [sum(sizes[:i]) for i in range(n)]
    return sizes, offs


def _build_phase1():
    nc = bacc.Bacc("TRN2", target_bir_lowering=False, debug=False,
                   num_devices=N_CORES)
    # host pre-arranges x/wgn so every DMA descriptor is one contiguous
    # multi-KB run per partition
    xh = nc.declare_dram_parameter("xh", [128, KD * T1], F32R, isOutput=False)
    wgnh = nc.declare_dram_parameter("wgnh", [128, KD * WGN], F32R,
                                     isOutput=False)
    bgn = nc.declare_dram_parameter("bgn", [WGN, 1], F32, isOutput=False)
    noiseT = nc.declare_dram_parameter("noiseT", [E, T1], F32, isOutput=False)
    route = nc.declare_dram_parameter("route", [128, NT * 4], F32,
                                      isOutput=True)

    with tile.TileContext(nc) as tc:
        with tc.tile_pool(name="sbuf", bufs=1) as pool, \
             tc.tile_pool(name="psum", bufs=1, space="PSUM") as psum:
            # one combined Exp+Ln table load, placed early with no deps
            nc.scalar.add_instruction(mybir.InstLoadActFuncSet(
                name=nc.get_next_instruction_name(),
                act_func_set_id=NLE_SET, ins=[], outs=[]))
            wsrc = pool.tile([1, 256], F32, tag="wsrc")
            nc.gpsimd.memset(wsrc[:], 1.0)
            ident = pool.tile([E, E], F32, tag="ident")
            make_identity(nc, ident[:])
            # PE warm-up during the x-DMA window
            wps = psum.tile([1, 256], F32, tag="wps")
            for w in range(6):
                nc.tensor.matmul(out=wps[:],
                                 lhsT=wsrc[0:1, 0:1].bitcast(F32R),
                                 rhs=wsrc[:].bitcast(F32R),
                                 start=True, stop=True)
            # x in 3 big pieces on the HWDGE (sync) queue; gating weights,
            # noise, biases behind them (all tiny)
            x_ps = []
            for p in range(3):
                xp = pool.tile([128, 2 * T1], F32R, tag=f"x{p}")
                nc.sync.dma_start(out=xp[:],
                                  in_=xh[:, p * 2 * T1:(p + 1) * 2 * T1])
                x_ps.append(xp)
            wgn_sb = pool.tile([128, KD * WGN], F32R, tag="wgn")
            nc.sync.dma_start(out=wgn_sb[:], in_=wgnh[:])
            noise_sb = pool.tile([E, T1], F32, tag="noise")
            nc.sync.dma_start(out=noise_sb[:], in_=noiseT[:])
            bgn_sb = pool.tile([WGN, 1], F32, tag="bgn")
            nc.sync.dma_start(out=bgn_sb[:], in_=bgn[:])

            # gating in transposed form: hps[e, tok], gate rows 0..7,
            # noise rows 32..39; accumulate as x pieces arrive
            hps = psum.tile([WGN, T1], F32, tag="hps")
            for k in range(KD):
                nc.tensor.matmul(out=hps[:],
                                 lhsT=wgn_sb[:, k * WGN:(k + 1) * WGN],
                                 rhs=x_ps[k // 2][:, (k % 2) * T1:
                                                  (k % 2) * T1 + T1],
                                 start=(k == 0), stop=(k == KD - 1))
            # softplus(z + bn) = ln(1 + exp(z + bn)) on the noise rows
            ex = pool.tile([E, T1], F32, tag="ex")
            nc.scalar.activation(ex[:], hps[32:32 + E, :], AF.Exp,
                                 bias=bgn_sb[32:32 + E, 0:1])
            sp = pool.tile([E, T1], F32, tag="sp")
            nc.scalar.activation(sp[:], ex[:], AF.Ln, bias=1.0)
            # gate rows + bg on DVE (runs during the ACT chain)
            hg = pool.tile([E, T1], F32, tag="hg")
            nc.vector.tensor_scalar_add(hg[:], hps[0:E, :],
                                        bgn_sb[0:E, 0:1])
            hfT = pool.tile([E, T1], F32, tag="hfT")
            nc.vector.tensor_mul(hfT[:], sp[:], noise_sb[:])
            nc.vector.tensor_add(hfT[:], hfT[:], hg[:])

            # token-major top-2 per 128-token tile
            mx = pool.tile([128, NT * 8], F32, tag="mx")
            ixa = pool.tile([128, NT * 8], U32, tag="ixa")
            tps = []
            for t in range(NT):
                tp = psum.tile([128, E], F32, tag=f"tp{t}")
                nc.tensor.transpose(out=tp[:],
                                    in_=hfT[:, t * 128:(t + 1) * 128],
                                    identity=ident[:])
                tps.append(tp)
            for t in range(NT):
                nc.vector.max(out=mx[:, t * 8:(t + 1) * 8], in_=tps[t][:])
            for t in range(NT):
                nc.vector.max_index(out=ixa[:, t * 8:(t + 1) * 8],
                                    in_max=mx[:, t * 8:(t + 1) * 8],
                                    in_values=tps[t][:])
            # pack [ix1 ix2 v1 v2] per token, DMA out partition-major
            ob = pool.tile([128, NT * 4], F32, tag="ob")
            mx3 = mx[:].rearrange("p (t e) -> p t e", t=NT)
            ix3 = ixa[:].rearrange("p (t e) -> p t e", t=NT)
            ob3 = ob[:].rearrange("p (t c) -> p t c", t=NT)
            nc.vector.tensor_copy(ob3[:, :, 0:2], ix3[:, :, 0:2])
            nc.vector.tensor_copy(ob3[:, :, 2:4], mx3[:, :, 0:2])
            nc.sync.dma_start(out=route[:], in_=ob[:])
    nc.compile()
    return nc


def _build_phase2(cap):
    tchs, tcho = _chunks_for(cap)
    nch = len(tchs)
    SL = KD * HSLAB  # 4608 cols per slab in the host-prearranged layouts
    PW = KD * 128    # 768 cols per (s, hh) piece of w1
    nc = bacc.Bacc("TRN2", target_bir_lowering=False, debug=False,
                   num_devices=N_CORES)
    # host pre-arranges all inputs partition-major so each DMA descriptor
    # is one contiguous multi-KB run per partition
    w1h = nc.declare_dram_parameter("w1h", [128, NSLAB * SL], BF16,
                                    isOutput=False)
    w2h = nc.declare_dram_parameter("w2h", [128, NSLAB * KH * D], BF16,
                                    isOutput=False)
    b1 = nc.declare_dram_parameter("b1", [128, H // 128], F32, isOutput=False)
    b2 = nc.declare_dram_parameter("b2", [128, ND], F32, isOutput=False)
    xh = nc.declare_dram_parameter("xh", [128, KD * cap], BF16,
                                   isOutput=False)
    g = nc.declare_dram_parameter("g", [128, cap], F32, isOutput=False)
    yT = nc.declare_dram_parameter("yT", [D, cap], BF16, isOutput=True)

    with tile.TileContext(nc) as tc:
        with tc.tile_pool(name="sbuf", bufs=2) as pool, \
             tc.tile_pool(name="sbig", bufs=1) as sbig, \
             tc.tile_pool(name="psum", bufs=2, space="PSUM") as psum:
            # PE warm-up first (before any DMA trigger occupies an engine):
            # dummy matmuls release the HAM clock throttle during the
            # input-DMA window
            wones = sbig.tile([1, 256], BF16, tag="wones")
            nc.gpsimd.memset(wones[:], 1.0)
            wps = psum.tile([1, 256], F32, tag="wps", name="wps")
            for w in range(14):
                nc.tensor.matmul(out=wps[:], lhsT=wones[0:1, 0:1],
                                 rhs=wones[:], start=True, stop=True)

            # Input DMAs in strict consumption order. Sync (HWDGE) carries
            # the stage-A critical path: first w1 piece, all of x, the rest
            # of w1. GpSimd (SWDGE) carries b1 then the stage-B inputs.
            # The Scalar queue carries NO DMA triggers so its table load
            # runs at t=0 and ACTIVATEs are never stuck behind a trigger.
            w1_sbs = [sbig.tile([128, SL], BF16, tag=f"w1_{s}",
                                name=f"w1_{s}") for s in range(NSLAB)]
            x_sb = sbig.tile([128, KD * cap], BF16, tag="x", name="x")
            nc.sync.dma_start(out=w1_sbs[0][:, 0:PW], in_=w1h[:, 0:PW])
            nc.sync.dma_start(out=x_sb[:], in_=xh[:])
            for hh in range(1, KH):
                nc.sync.dma_start(out=w1_sbs[0][:, hh * PW:(hh + 1) * PW],
                                  in_=w1h[:, hh * PW:(hh + 1) * PW])
            for s in range(1, NSLAB):
                nc.sync.dma_start(out=w1_sbs[s][:],
                                  in_=w1h[:, s * SL:(s + 1) * SL])
            b1_sb = sbig.tile([128, H // 128], F32, tag="b1")
            nc.gpsimd.dma_start(out=b1_sb[:], in_=b1[:])
            w2_sbs = []
            for s in range(NSLAB):
                w2_sb = sbig.tile([128, KH * D], BF16, tag=f"w2_{s}")
                nc.gpsimd.dma_start(out=w2_sb[:],
                                    in_=w2h[:, s * KH * D:(s + 1) * KH * D])
                w2_sbs.append(w2_sb)
            b2_sb = sbig.tile([128, ND], F32, tag="b2")
            nc.gpsimd.dma_start(out=b2_sb[:], in_=b2[:])
            g_sb = sbig.tile([128, cap], F32, tag="g")
            nc.gpsimd.dma_start(out=g_sb[:], in_=g[:])
            hid_sbs = [sbig.tile([128, KH * cap], BF16, tag=f"hid_{s}",
                                 name=f"hid_{s}")
                       for s in range(NSLAB)]

            def x_k(k, lo, hi):
                return x_sb[:, k * cap + lo:k * cap + hi]

            # Stage A: hid = relu(x@W1 + b1), full H materialized in SBUF
            for s in range(NSLAB):
                for hh in range(KH):
                    pst = [psum.tile([128, tchs[i]], F32, tag=f"ps{i}",
                                     name=f"psA_{s}_{hh}_{i}")
                           for i in range(nch)]
                    for k in range(KD):
                        for i in range(nch):
                            nc.tensor.matmul(
                                out=pst[i][:],
                                lhsT=w1_sbs[s][:, hh * PW + k * 128:
                                               hh * PW + k * 128 + 128],
                                rhs=x_k(k, tcho[i], tcho[i] + tchs[i]),
                                start=(k == 0), stop=(k == KD - 1))
                    for i in range(nch):
                        nc.scalar.activation(
                            hid_sbs[s][:, hh * cap + tcho[i]:
                                       hh * cap + tcho[i] + tchs[i]],
                            pst[i][:], AF.Relu,
                            bias=b1_sb[:, s * KH + hh:s * KH + hh + 1])

            # Stage B: y = (hid@W2 + b2) * g, contracting the full H in PSUM
            for dt in range(ND):
                psy = [psum.tile([128, tchs[i]], F32, tag=f"ps{i}",
                                 name=f"psB_{dt}_{i}")
                       for i in range(nch)]
                for j in range(NSLAB * KH):
                    s, jj = divmod(j, KH)
                    for i in range(nch):
                        nc.tensor.matmul(
                            out=psy[i][:],
                            lhsT=w2_sbs[s][:, jj * D + dt * 128:
                                           jj * D + dt * 128 + 128],
                            rhs=hid_sbs[s][:, jj * cap + tcho[i]:
                                           jj * cap + tcho[i] + tchs[i]],
                            start=(j == 0), stop=(j == NSLAB * KH - 1))
                yp = pool.tile([128, cap], F32, tag="yp", name=f"yp_{dt}")
                yo = pool.tile([128, cap], BF16, tag="yo", name=f"yo_{dt}")
                for i in range(nch):
                    sl = slice(tcho[i], tcho[i] + tchs[i])
                    nc.scalar.activation(yp[:, sl], psy[i][:], AF.Identity,
                                         bias=b2_sb[:, dt:dt + 1])
                    nc.vector.tensor_mul(yo[:, sl], yp[:, sl], g_sb[:, sl])
                    # sync/gpsimd only: a trigger on the scalar queue would
                    # delay the next chunk's ACT in its FIFO
                    (nc.sync if i % 2 == 0 else nc.gpsimd).dma_start(
                        out=yT[dt * 128:(dt + 1) * 128, sl], in_=yo[:, sl])
    nc.compile()
    return nc


def kernel(x, noise, Wg, bg, Wn, bn, W1, b1, W2, b2):
    x = np.asarray(x, dtype=np.float32)
    noise = np.asarray(noise, dtype=np.float32)
    Wg = np.asarray(Wg, dtype=np.float32)
    bg = np.asarray(bg, dtype=np.float32)
    Wn = np.asarray(Wn, dtype=np.float32)
    bn = np.asarray(bn, dtype=np.float32)
    W1 = np.asarray(W1, dtype=np.float32)
    b1 = np.asarray(b1, dtype=np.float32)
    W2 = np.asarray(W2, dtype=np.float32)
    b2 = np.asarray(b2, dtype=np.float32)

    if "p1" not in _cache:
        _cache["p1"] = _build_phase1()

    x2d = x.reshape(T, D)
    xT = np.ascontiguousarray(x2d.T)                      # [D, T]
    n2d = noise.reshape(T, E)
    wgn = np.zeros((D, WGN), dtype=np.float32)  # gate cols 0..7, noise 32..39
    wgn[:, 0:E] = Wg
    wgn[:, 32:32 + E] = Wn
    bgn = np.zeros((WGN, 1), dtype=np.float32)
    bgn[0:E, 0] = bg
    bgn[32:32 + E, 0] = bn

    # ── Phase 1: gating (token-sharded) ──
    # partition-major layouts: row p holds that partition's full k-range
    wgnh = np.ascontiguousarray(
        wgn.reshape(KD, 128, WGN).transpose(1, 0, 2).reshape(128, KD * WGN))
    in_maps1 = [{
        "xh": np.ascontiguousarray(
            xT[:, c * T1:(c + 1) * T1].reshape(KD, 128, T1)
            .transpose(1, 0, 2).reshape(128, KD * T1)),
        "wgnh": wgnh,
        "bgn": bgn,
        "noiseT": np.ascontiguousarray(n2d[c * T1:(c + 1) * T1, :].T),
    } for c in range(N_CORES)]
    res1 = run_bass_kernel_spmd(_cache["p1"], in_maps1,
                                core_ids=list(range(N_CORES)))
    # route rows are partitions, cols (t*4 + c); token = t*128 + p
    route = np.concatenate([
        res1.results[c]["route"].reshape(128, NT, 4).transpose(1, 0, 2)
        .reshape(T1, 4)
        for c in range(N_CORES)], axis=0)                  # [T, 4]
    last_perf["p1"] = res1.exec_time_ns
    if res1.instructions_and_trace:
        last_perf["p1_insts"] = res1.instructions_and_trace[0]

    a1 = route[:, 0].astype(np.int64)
    a2 = route[:, 1].astype(np.int64)
    v1 = route[:, 2].astype(np.float64)
    v2 = route[:, 3].astype(np.float64)
    q = np.exp(v2 - v1)                                    # ≤ 1
    p1 = (1.0 / (1.0 + q)).astype(np.float32)
    p2 = (1.0 - p1).astype(np.float32)

    # ── Host dispatch: gather tokens per expert ──
    idxs, gates = [], []
    maxload = 0
    for e in range(E):
        m1 = a1 == e
        m2 = a2 == e
        idx = np.nonzero(m1 | m2)[0]
        gv = np.where(m1, p1, p2)[idx]
        idxs.append(idx)
        gates.append(gv)
        maxload = max(maxload, idx.size)

    cap = CAP if maxload <= CAP else -(-maxload // 96) * 96
    key = ("p2", cap)
    if key not in _cache:
        _cache[key] = _build_phase2(cap)

    xT_bf = xT.astype(BFNP)
    in_maps2 = []
    for e in range(E):
        idx = idxs[e]
        xc = np.zeros((D, cap), dtype=BFNP)
        xc[:, :idx.size] = xT_bf[:, idx]
        gv = np.zeros((cap,), dtype=np.float32)
        gv[:idx.size] = gates[e]
        # partition-major layouts (see _build_phase2); w1h is (s, hh, k, c)
        w1h = (W1[e].astype(BFNP).reshape(KD, 128, NSLAB, KH, 128)
               .transpose(1, 2, 3, 0, 4).reshape(128, NSLAB * KD * HSLAB))
        w2h = (W2[e].astype(BFNP).reshape(NSLAB, KH, 128, D)
               .transpose(2, 0, 1, 3).reshape(128, NSLAB * KH * D))
        xh = (xc.reshape(KD, 128, cap).transpose(1, 0, 2)
              .reshape(128, KD * cap))
        in_maps2.append({
            "w1h": np.ascontiguousarray(w1h),
            "w2h": np.ascontiguousarray(w2h),
            "b1": np.ascontiguousarray(b1[e].reshape(H // 128, 128).T),
            "b2": np.ascontiguousarray(b2[e].reshape(ND, 128).T),
            "xh": np.ascontiguousarray(xh),
            "g": np.ascontiguousarray(np.broadcast_to(gv, (128, cap))),
        })
    res2 = run_bass_kernel_spmd(_cache[key], in_maps2,
                                core_ids=list(range(N_CORES)))
    last_perf["p2"] = res2.exec_time_ns
    if res2.instructions_and_trace:
        last_perf["p2_insts"] = res2.instructions_and_trace[0]

    # ── Host combine: scatter-add per-expert outputs ──
    out = np.zeros((T, D), dtype=np.float32)
    for e in range(E):
        idx = idxs[e]
        yT_ = res2.results[e]["yT"]                        # [D, cap] bf16
        out[idx] += yT_[:, :idx.size].T.astype(np.float32)
    return out.reshape(B, S, D)
